# revision 13
# baseline (speedup 1.0000x reference)
"""Distributed Trainium2 kernel for nn_Attention_30494267801907.

Multi-head attention (H=16, D=64, N=4096) with RoPE + QK-L2-norm + learned
qk_scale, softmax, and output projection, tensor-parallel over heads on 8
NeuronCores (2 heads per core).

Per-core pipeline (layouts chosen so nothing on the N^2 path needs a
transpose or an extra DVE pass):
  1. prep in natural [seq, feat] bf16, batched in 8-tile groups: rope via
     x*cos + swap(x)*ss with qk_scale^2 pre-folded into the k tables on
     the host; L2 norms from raw x (rope preserves norms); rsqrt = one
     batched ACT-sqrt + DVE reciprocal; the 1/norm scale is applied with
     a free-dim step-0 broadcast AP in one op per group. PE-transposes
     128x128 blocks into [feat, seq]; k before q (attention consumes all
     of k in its first i-chunk).
  2. scores transposed: S^T[j,i] = k^T(j).q^T(i) via K=64 matmuls, both
     heads packed in disjoint PE row-halves (concurrent).
  3. exp on ACT straight from PSUM (softmax 1/sqrt(D) folded into the
     activation scale), bf16 out. Scores are bounded by qk_scale^2/8 so
     no max-subtraction pass is needed.
  4. PV with stationary [v | 1x64]: out^T[d,i] accumulates over j-blocks
     in PSUM; rows 64..127 accumulate the softmax denominator, so 1/den
     arrives partition-parallel with no broadcast.
  5. two i-chunks run interleaved (A/B) so the PE always has QK/PV work
     while ACT computes the other chunk's exp - keeps the PE HAM-warm.
  6. AllGather A^T in 512-column chunks right after each i-chunk
     finishes (overlaps remaining attention), then the projection
     y^T[o_local, i] = wT^T @ A^T + bias per chunk.
Host concatenates the per-core y^T column slices and transposes.
"""

import math
from contextlib import ExitStack

import numpy as np
import ml_dtypes

import concourse.bass as bass
import concourse.mybir as mybir
import concourse.tile as tile
from concourse import bacc
from concourse.bass import ds, ts
from concourse.masks import make_identity

F32 = mybir.dt.float32
BF16 = mybir.dt.bfloat16

N_CORES = 8
SEQ = 4096
D = 64          # head dim
HL = 2          # heads per core
FL = HL * D     # local feature cols (128)
DIMF = 1024     # full feature dim
OC = DIMF // N_CORES  # output cols per core (128)
P = 128         # partition tile


def build_graph(seq=SEQ, n_cores=N_CORES):
    nc = bacc.Bacc("TRN2", target_bir_lowering=False, debug=False,
                   num_devices=n_cores)
    NT = seq // P            # seq tiles == j blocks
    IC = min(512, seq)       # query-chunk per softmax pass
    NIC = seq // IC
    SUBW = min(512, IC)      # matmul moving-operand width
    NSUB = IC // SUBW
    KB = DIMF // P           # K blocks in projection
    GT = min(8, NT)          # seq tiles per prep group
    NG = NT // GT
    CH = min(8, NIC)         # all-gather/projection chunks
    ICPC = NIC // CH         # i-chunks per gather chunk
    hseq = seq // CH
    PAIRED = NIC % 2 == 0 and NIC >= 2

    q_d = nc.declare_dram_parameter("q", [seq, FL], BF16, isOutput=False)
    k_d = nc.declare_dram_parameter("k", [seq, FL], BF16, isOutput=False)
    v_d = nc.declare_dram_parameter("v", [seq, FL], BF16, isOutput=False)
    cq_d = nc.declare_dram_parameter("cosq", [seq, FL], BF16, isOutput=False)
    sq_d = nc.declare_dram_parameter("ssq", [seq, FL], BF16, isOutput=False)
    ck_d = nc.declare_dram_parameter("cosk", [seq, FL], BF16, isOutput=False)
    sk_d = nc.declare_dram_parameter("ssk", [seq, FL], BF16, isOutput=False)
    wt_d = nc.declare_dram_parameter("wt", [DIMF, OC], BF16, isOutput=False)
    b_d = nc.declare_dram_parameter("bias", [OC, 1], F32, isOutput=False)
    out_d = nc.declare_dram_parameter("out", [OC, seq], F32, isOutput=True)

    def grp(dram):
        # [seq, FL] -> [P, NG, GT, FL] view: partition = row-in-tile
        return dram[:, :].rearrange("(g a p) f -> p g a f", p=P, a=GT)

    with ExitStack() as ctx:
        tc = ctx.enter_context(tile.TileContext(nc))

        const = ctx.enter_context(tc.tile_pool(name="const", bufs=1))
        big = ctx.enter_context(tc.tile_pool(name="big", bufs=1))
        nrm = ctx.enter_context(tc.tile_pool(name="nrm", bufs=1))
        dram = ctx.enter_context(tc.tile_pool(name="dram", bufs=1,
                                              space="DRAM"))

        ident = const.tile([P, P], BF16)
        make_identity(nc, ident)
        bias_sb = const.tile([OC, 1], F32)
        nc.sync.dma_start(out=bias_sb, in_=b_d[:, :])
        wt_sb = const.tile([P, KB, OC], BF16)
        nc.sync.dma_start(
            out=wt_sb, in_=wt_d[:, :].rearrange("(kb p) o -> p kb o", p=P))

        # persistent operands
        qT = big.tile([P, seq], BF16)          # [f_local, i]
        kT = big.tile([P, seq], BF16)          # [f_local, j]
        v1A = big.tile([P, NT, 2 * D], BF16)   # [j_in_blk, jb, d | ones]
        v1B = big.tile([P, NT, 2 * D], BF16)
        aT = big.tile([P, seq], BF16)          # normalized attn out^T

        nc.vector.memset(v1A, 1.0)
        nc.vector.memset(v1B, 1.0)

        ssq_q = nrm.tile([P, NT, HL], F32)
        ssq_k = nrm.tile([P, NT, HL], F32)
        srt_q = nrm.tile([P, NT, HL], F32)
        srt_k = nrm.tile([P, NT, HL], F32)
        rn_q = nrm.tile([P, NT, HL], F32)
        rn_k = nrm.tile([P, NT, HL], F32)

        def swap_pairs(ap):
            # view with each (2i, 2i+1) free-dim pair swapped
            return bass.AP(tensor=ap.tensor, offset=ap.offset + 1,
                           ap=list(ap.ap[:-1]) + [[2, ap.ap[-1][1] // 2],
                                                  [-1, 2]])

        def rn_bcast(rn, g):
            # [P, GT, HL] slice broadcast over D along free dim
            sl = rn[:, ds(g * GT, GT), :]
            return bass.AP(tensor=sl.tensor, offset=sl.offset,
                           ap=list(sl.ap) + [[0, D]])

        # ---------------- prep ----------------
        with tc.tile_pool(name="prep", bufs=2) as prep, \
             tc.tile_pool(name="tp_psum", bufs=2, space="PSUM") as tpp:

            def prep_tensor(x_d, c_d, s_d, ssq, srt, rn, dstT, tag):
                xX = prep.tile([P, NT, FL], BF16, tag="xX", name=f"xX{tag}")
                cX = prep.tile([P, NT, FL], BF16, tag="cX", name=f"cX{tag}")
                sX = prep.tile([P, NT, FL], BF16, tag="sX", name=f"sX{tag}")
                xS = prep.tile([P, NT, FL], BF16, tag="xS", name=f"xS{tag}")
                for g in range(NG):
                    gs = ds(g * GT, GT)
                    nc.sync.dma_start(out=xX[:, gs, :], in_=grp(x_d)[:, g])
                    nc.sync.dma_start(out=cX[:, gs, :], in_=grp(c_d)[:, g])
                    nc.sync.dma_start(out=sX[:, gs, :], in_=grp(s_d)[:, g])
                for g in range(NG):
                    gs = ds(g * GT, GT)
                    tmp = prep.tile([P, GT, FL], BF16, tag="tmp",
                                    name=f"tmp{tag}_{g}")
                    nc.vector.tensor_mul(tmp, xX[:, gs, :], xX[:, gs, :])
                    nc.vector.tensor_reduce(
                        ssq[:, gs, :],
                        tmp.rearrange("p a (h d) -> p a h d", h=HL),
                        axis=mybir.AxisListType.X, op=mybir.AluOpType.add)
                nc.scalar.sqrt(srt, ssq)
                nc.vector.reciprocal(rn, srt)
                for g in range(NG):
                    gs = ds(g * GT, GT)
                    ra = prep.tile([P, GT, FL], BF16, tag="rra",
                                   name=f"ra{tag}_{g}")
                    rb = prep.tile([P, GT, FL], BF16, tag="rrb",
                                   name=f"rb{tag}_{g}")
                    nc.vector.tensor_mul(ra, xX[:, gs, :], cX[:, gs, :])
                    nc.vector.tensor_mul(rb, swap_pairs(xX[:, gs, :]),
                                         sX[:, gs, :])
                    nc.vector.tensor_add(ra, ra, rb)
                    nc.vector.tensor_mul(
                        xS[:, gs, :].rearrange("p a (h d) -> p a h d", h=HL),
                        ra.rearrange("p a (h d) -> p a h d", h=HL),
                        rn_bcast(rn, g))
                    for a in range(GT):
                        t = g * GT + a
                        tp = tpp.tile([P, P], BF16, tag="tp",
                                      name=f"tp{tag}_{t}")
                        nc.tensor.transpose(tp, xS[:, t, :], ident)
                        nc.scalar.copy(dstT[:, ts(t, P)], tp)

            # k first: attention consumes every k block in its first pass
            prep_tensor(k_d, ck_d, sk_d, ssq_k, srt_k, rn_k, kT, "k")
            prep_tensor(q_d, cq_d, sq_d, ssq_q, srt_q, rn_q, qT, "q")

            for g in range(NG):
                vg = prep.tile([P, GT, FL], BF16, tag="vg", name=f"vg_{g}")
                nc.sync.dma_start(out=vg, in_=grp(v_d)[:, g])
                gs = ds(g * GT, GT)
                nc.vector.tensor_copy(v1A[:, gs, 0:D], vg[:, :, 0:D])
                nc.vector.tensor_copy(v1B[:, gs, 0:D], vg[:, :, D:FL])

        # ------------- attention + gather + projection -------------
        cc_in = []
        cc_out = []
        for hf in range(CH):
            cin_t = dram.tile([FL, hseq], BF16, name=f"cc_in{hf}")
            cout_t = dram.tile([DIMF, hseq], BF16, addr_space="Shared",
                               name=f"cc_out{hf}")
            cc_in.append(cin_t)
            cc_out.append(cout_t)

        with tc.tile_pool(name="s_psum", bufs=2, space="PSUM") as spool, \
             tc.tile_pool(name="o_psum", bufs=4, space="PSUM") as opool, \
             tc.tile_pool(name="epool", bufs=4) as epool, \
             tc.tile_pool(name="rpool", bufs=2) as rpool, \
             tc.tile_pool(name="agp", bufs=2) as agp, \
             tc.tile_pool(name="ypool", bufs=3) as ypool:

            def emit_gather_proj(chunk):
                nc.sync.dma_start(out=cc_in[chunk][:, :],
                                  in_=aT[:, ds(chunk * hseq, hseq)])
                nc.gpsimd.collective_compute(
                    "AllGather", mybir.AluOpType.bypass,
                    replica_groups=[list(range(n_cores))],
                    ins=[cc_in[chunk].opt()], outs=[cc_out[chunk].opt()])
                ag = []
                for kb in range(KB):
                    t_ = agp.tile([P, hseq], BF16, tag=f"ag{kb}",
                                  name=f"ag{chunk}_{kb}")
                    nc.sync.dma_start(out=t_,
                                      in_=cc_out[chunk][ds(kb * P, P), :])
                    ag.append(t_)
                for nb in range(hseq // SUBW):
                    # projection PSUM borrows a score-pool slot
                    py = spool.tile([OC, SUBW], F32, tag="sAB",
                                    name=f"py{chunk}_{nb}")
                    for kb in range(KB):
                        nc.tensor.matmul(
                            py, lhsT=wt_sb[:, kb, :],
                            rhs=ag[kb][:, ts(nb, SUBW)],
                            start=(kb == 0), stop=(kb == KB - 1))
                    ysb = ypool.tile([OC, SUBW], F32, tag="ysb",
                                     name=f"ysb{chunk}_{nb}")
                    nc.vector.tensor_scalar_add(ysb, py, bias_sb)
                    nc.sync.dma_start(
                        out=out_d[:, ds(chunk * hseq + nb * SUBW, SUBW)],
                        in_=ysb)

            def emit_qk(ic, jb):
                sAB = spool.tile([P, 2 * IC], F32, tag="sAB",
                                 name=f"sAB_{ic}_{jb}")
                for h, cofs in ((0, 0), (1, IC)):
                    hd = ds(h * D, D)
                    for sub in range(NSUB):
                        nc.tensor.matmul(
                            sAB[:, ds(cofs + sub * SUBW, SUBW)],
                            lhsT=kT[hd, ts(jb, P)],
                            rhs=qT[hd, ds(ic * IC + sub * SUBW, SUBW)],
                            start=True, stop=True)
                return sAB

            step = 2 if PAIRED else 1
            for icp in range(0, NIC, step):
                group = [icp + i for i in range(step)]
                otiles = {}
                for ic in group:
                    otiles[ic] = (
                        opool.tile([P, IC], F32, tag="o", name=f"oA_{ic}"),
                        opool.tile([P, IC], F32, tag="o", name=f"oB_{ic}"))
                s_cur = {ic: emit_qk(ic, 0) for ic in group}
                for jb in range(NT):
                    for ic in group:
                        eAB = epool.tile([P, 2 * IC], BF16, tag="e",
                                         name=f"e_{ic}_{jb}")
                        nc.scalar.activation(
                            eAB, s_cur[ic], mybir.ActivationFunctionType.Exp,
                            scale=1.0 / math.sqrt(D))
                        if jb + 1 < NT:
                            s_cur[ic] = emit_qk(ic, jb + 1)
                        oA, oB = otiles[ic]
                        for cofs, op_, v1 in ((0, oA, v1A), (IC, oB, v1B)):
                            for sub in range(NSUB):
                                nc.tensor.matmul(
                                    op_[:, ds(sub * SUBW, SUBW)],
                                    lhsT=v1[:, jb, :],
                                    rhs=eAB[:, ds(cofs + sub * SUBW, SUBW)],
                                    start=(jb == 0), stop=(jb == NT - 1))
                for ic in group:
                    oA, oB = otiles[ic]
                    for h, op_ in ((0, oA), (1, oB)):
                        rec_b = rpool.tile([D, IC], F32, tag="rec",
                                           name=f"rec_{ic}_{h}")
                        nc.vector.reciprocal(rec_b, op_[D:2 * D, :])
                        nc.vector.tensor_mul(
                            aT[ds(h * D, D), ds(ic * IC, IC)],
                            op_[0:D, :], rec_b)
                    if (ic + 1) % ICPC == 0:
                        emit_gather_proj((ic + 1) // ICPC - 1)

    nc.compile()
    return nc


def host_inputs(q, k, v, qk_scale, w_out, b_out, n_cores=N_CORES):
    """Shard + derive per-core input maps from the full problem inputs."""
    B, N, dim = q.shape
    assert B == 1 and dim == DIMF
    bf16 = ml_dtypes.bfloat16

    inv_freq = 1.0 / (10000.0 ** (np.arange(0, D, 2, dtype=np.float64) / D))
    t = np.arange(N, dtype=np.float64)
    freqs = np.outer(t, inv_freq)                       # [N, D/2]
    cos_e = np.repeat(np.cos(freqs), 2, axis=1)         # [N, D]
    ss_e = np.empty((N, D), dtype=np.float64)
    ss_e[:, 0::2] = -np.sin(freqs)
    ss_e[:, 1::2] = np.sin(freqs)
    sv = qk_scale.reshape(-1).astype(np.float64) ** 2   # [D]
    cosq = np.tile(cos_e, (1, HL)).astype(bf16)         # [N, FL]
    ssq = np.tile(ss_e, (1, HL)).astype(bf16)
    cosk = np.tile(cos_e * sv, (1, HL)).astype(bf16)
    ssk = np.tile(ss_e * sv, (1, HL)).astype(bf16)

    in_maps = []
    for c in range(n_cores):
        sl = slice(FL * c, FL * (c + 1))
        in_maps.append({
            "q": np.ascontiguousarray(q[0, :, sl]).astype(bf16),
            "k": np.ascontiguousarray(k[0, :, sl]).astype(bf16),
            "v": np.ascontiguousarray(v[0, :, sl]).astype(bf16),
            "cosq": cosq,
            "ssq": ssq,
            "cosk": cosk,
            "ssk": ssk,
            "wt": np.ascontiguousarray(w_out[sl, :].T).astype(bf16),
            "bias": np.ascontiguousarray(
                b_out[sl].reshape(OC, 1), dtype=np.float32),
        })
    return in_maps


def assemble_output(results, N=SEQ, n_cores=N_CORES):
    out = np.empty((1, N, DIMF), dtype=np.float32)
    for c in range(n_cores):
        out[0, :, FL * c:FL * (c + 1)] = results[c]["out"].T
    return out


_CACHE = {}


def kernel(q, k, v, qk_scale, w_out, b_out):
    from concourse.bass_utils import run_bass_kernel_spmd

    if "nc" not in _CACHE:
        _CACHE["nc"] = build_graph()
    nc = _CACHE["nc"]
    in_maps = host_inputs(q, k, v, qk_scale, w_out, b_out)
    res = run_bass_kernel_spmd(nc, in_maps, core_ids=list(range(N_CORES)))
    return assemble_output(res.results)


# revision 15
# speedup vs baseline: 1.2945x; 1.2945x over previous
"""Distributed Trainium2 kernel for nn_Attention_30494267801907.

Multi-head attention (H=16, D=64, N=4096) with RoPE + QK-L2-norm + learned
qk_scale, softmax, and output projection, tensor-parallel over heads on 8
NeuronCores (2 heads per core).

Per-core pipeline (layouts chosen so nothing on the N^2 path needs a
transpose or an extra DVE pass):
  1. prep in natural [seq, feat] bf16, batched in 8-tile groups: rope via
     x*cos + swap(x)*ss with qk_scale^2 pre-folded into the k tables on
     the host; L2 norms from raw x (rope preserves norms); rsqrt = one
     batched ACT-sqrt + DVE reciprocal; the 1/norm scale is applied with
     a free-dim step-0 broadcast AP in one op per group. PE-transposes
     128x128 blocks into [feat, seq]; k before q (attention consumes all
     of k in its first i-chunk).
  2. scores transposed: S^T[j,i] = k^T(j).q^T(i) via K=64 matmuls, both
     heads packed in disjoint PE row-halves (concurrent).
  3. exp on ACT straight from PSUM (softmax 1/sqrt(D) folded into the
     activation scale), bf16 out. Scores are bounded by qk_scale^2/8 so
     no max-subtraction pass is needed.
  4. PV with stationary [v | 1x64]: out^T[d,i] accumulates over j-blocks
     in PSUM; rows 64..127 accumulate the softmax denominator, so 1/den
     arrives partition-parallel with no broadcast.
  5. two i-chunks run interleaved (A/B) so the PE always has QK/PV work
     while ACT computes the other chunk's exp - keeps the PE HAM-warm.
  6. AllGather A^T in 512-column chunks right after each i-chunk
     finishes (overlaps remaining attention), then the projection
     y^T[o_local, i] = wT^T @ A^T + bias per chunk.
Host concatenates the per-core y^T column slices and transposes.
"""

import math
from contextlib import ExitStack

import numpy as np
import ml_dtypes

import concourse.bass as bass
import concourse.mybir as mybir
import concourse.tile as tile
from concourse import bacc
from concourse.bass import ds, ts
from concourse.masks import make_identity

F32 = mybir.dt.float32
BF16 = mybir.dt.bfloat16

N_CORES = 8
SEQ = 4096
D = 64          # head dim
HL = 2          # heads per core
FL = HL * D     # local feature cols (128)
DIMF = 1024     # full feature dim
OC = DIMF // N_CORES  # output cols per core (128)
P = 128         # partition tile


def build_graph(seq=SEQ, n_cores=N_CORES):
    nc = bacc.Bacc("TRN2", target_bir_lowering=False, debug=False,
                   num_devices=n_cores)
    NT = seq // P            # seq tiles == j blocks
    IC = min(512, seq)       # query-chunk per softmax pass
    NIC = seq // IC
    SUBW = min(512, IC)      # matmul moving-operand width
    NSUB = IC // SUBW
    KB = DIMF // P           # K blocks in projection
    GT = min(8, NT)          # seq tiles per prep group
    NG = NT // GT
    CH = min(8, NIC)         # all-gather/projection chunks
    ICPC = NIC // CH         # i-chunks per gather chunk
    hseq = seq // CH
    PAIRED = NIC % 2 == 0 and NIC >= 2

    q_d = nc.declare_dram_parameter("q", [seq, FL], BF16, isOutput=False)
    k_d = nc.declare_dram_parameter("k", [seq, FL], BF16, isOutput=False)
    v_d = nc.declare_dram_parameter("v", [seq, FL], BF16, isOutput=False)
    cq_d = nc.declare_dram_parameter("cosq", [seq, FL], BF16, isOutput=False)
    sq_d = nc.declare_dram_parameter("ssq", [seq, FL], BF16, isOutput=False)
    ck_d = nc.declare_dram_parameter("cosk", [seq, FL], BF16, isOutput=False)
    sk_d = nc.declare_dram_parameter("ssk", [seq, FL], BF16, isOutput=False)
    wt_d = nc.declare_dram_parameter("wt", [DIMF, OC], BF16, isOutput=False)
    b_d = nc.declare_dram_parameter("bias", [OC, 1], F32, isOutput=False)
    out_d = nc.declare_dram_parameter("out", [OC, seq], F32, isOutput=True)

    def grp(dram):
        # [seq, FL] -> [P, NG, GT, FL] view: partition = row-in-tile
        return dram[:, :].rearrange("(g a p) f -> p g a f", p=P, a=GT)

    with ExitStack() as ctx:
        tc = ctx.enter_context(tile.TileContext(nc))

        const = ctx.enter_context(tc.tile_pool(name="const", bufs=1))
        big = ctx.enter_context(tc.tile_pool(name="big", bufs=1))
        nrm = ctx.enter_context(tc.tile_pool(name="nrm", bufs=1))
        dram = ctx.enter_context(tc.tile_pool(name="dram", bufs=1,
                                              space="DRAM"))

        ident = const.tile([P, P], BF16)
        make_identity(nc, ident)
        bias_sb = const.tile([OC, 1], F32)
        nc.sync.dma_start(out=bias_sb, in_=b_d[:, :])
        wt_sb = const.tile([P, KB, OC], BF16)
        nc.sync.dma_start(
            out=wt_sb, in_=wt_d[:, :].rearrange("(kb p) o -> p kb o", p=P))

        # persistent operands
        qT = big.tile([P, seq], BF16)          # [f_local, i]
        kT = big.tile([P, seq], BF16)          # [f_local, j]
        v1A = big.tile([P, NT, 2 * D], BF16)   # [j_in_blk, jb, d | ones]
        v1B = big.tile([P, NT, 2 * D], BF16)
        aT = big.tile([P, seq], BF16)          # normalized attn out^T

        nc.vector.memset(v1A, 1.0)
        nc.vector.memset(v1B, 1.0)

        ssq_q = nrm.tile([P, NT, HL], F32)
        ssq_k = nrm.tile([P, NT, HL], F32)
        srt_q = nrm.tile([P, NT, HL], F32)
        srt_k = nrm.tile([P, NT, HL], F32)
        rn_q = nrm.tile([P, NT, HL], F32)
        rn_k = nrm.tile([P, NT, HL], F32)

        def swap_pairs(ap):
            # view with each (2i, 2i+1) free-dim pair swapped
            return bass.AP(tensor=ap.tensor, offset=ap.offset + 1,
                           ap=list(ap.ap[:-1]) + [[2, ap.ap[-1][1] // 2],
                                                  [-1, 2]])

        def rn_bcast(rn, g):
            # [P, GT, HL] slice broadcast over D along free dim
            sl = rn[:, ds(g * GT, GT), :]
            return bass.AP(tensor=sl.tensor, offset=sl.offset,
                           ap=list(sl.ap) + [[0, D]])

        # ---------------- prep ----------------
        with tc.tile_pool(name="prep", bufs=2) as prep, \
             tc.tile_pool(name="tp_psum", bufs=2, space="PSUM") as tpp:

            def prep_tensor(x_d, c_d, s_d, ssq, srt, rn, dstT, tag):
                xX = prep.tile([P, NT, FL], BF16, tag="xX", name=f"xX{tag}")
                cX = prep.tile([P, NT, FL], BF16, tag="cX", name=f"cX{tag}")
                sX = prep.tile([P, NT, FL], BF16, tag="sX", name=f"sX{tag}")
                xS = prep.tile([P, NT, FL], BF16, tag="xS", name=f"xS{tag}")
                for g in range(NG):
                    gs = ds(g * GT, GT)
                    nc.sync.dma_start(out=xX[:, gs, :], in_=grp(x_d)[:, g])
                    nc.sync.dma_start(out=cX[:, gs, :], in_=grp(c_d)[:, g])
                    nc.sync.dma_start(out=sX[:, gs, :], in_=grp(s_d)[:, g])
                for g in range(NG):
                    gs = ds(g * GT, GT)
                    tmp = prep.tile([P, GT, FL], BF16, tag="tmp",
                                    name=f"tmp{tag}_{g}")
                    nc.vector.tensor_mul(tmp, xX[:, gs, :], xX[:, gs, :])
                    nc.vector.tensor_reduce(
                        ssq[:, gs, :],
                        tmp.rearrange("p a (h d) -> p a h d", h=HL),
                        axis=mybir.AxisListType.X, op=mybir.AluOpType.add)
                nc.scalar.sqrt(srt, ssq)
                nc.vector.reciprocal(rn, srt)
                for g in range(NG):
                    gs = ds(g * GT, GT)
                    ra = prep.tile([P, GT, FL], BF16, tag="rra",
                                   name=f"ra{tag}_{g}")
                    rb = prep.tile([P, GT, FL], BF16, tag="rrb",
                                   name=f"rb{tag}_{g}")
                    nc.vector.tensor_mul(ra, xX[:, gs, :], cX[:, gs, :])
                    nc.vector.tensor_mul(rb, swap_pairs(xX[:, gs, :]),
                                         sX[:, gs, :])
                    nc.vector.tensor_add(ra, ra, rb)
                    nc.vector.tensor_mul(
                        xS[:, gs, :].rearrange("p a (h d) -> p a h d", h=HL),
                        ra.rearrange("p a (h d) -> p a h d", h=HL),
                        rn_bcast(rn, g))
                    for a in range(GT):
                        t = g * GT + a
                        tp = tpp.tile([P, P], BF16, tag="tp",
                                      name=f"tp{tag}_{t}")
                        nc.tensor.transpose(tp, xS[:, t, :], ident)
                        nc.scalar.copy(dstT[:, ts(t, P)], tp)

            # k first: attention consumes every k block in its first pass
            prep_tensor(k_d, ck_d, sk_d, ssq_k, srt_k, rn_k, kT, "k")
            prep_tensor(q_d, cq_d, sq_d, ssq_q, srt_q, rn_q, qT, "q")

            for g in range(NG):
                vg = prep.tile([P, GT, FL], BF16, tag="vg", name=f"vg_{g}")
                nc.sync.dma_start(out=vg, in_=grp(v_d)[:, g])
                gs = ds(g * GT, GT)
                nc.vector.tensor_copy(v1A[:, gs, 0:D], vg[:, :, 0:D])
                nc.vector.tensor_copy(v1B[:, gs, 0:D], vg[:, :, D:FL])

        # ------------- attention + gather + projection -------------
        cc_in = []
        cc_out = []
        for hf in range(CH):
            cin_t = dram.tile([FL, hseq], BF16, name=f"cc_in{hf}")
            cout_t = dram.tile([DIMF, hseq], BF16, addr_space="Shared",
                               name=f"cc_out{hf}")
            cc_in.append(cin_t)
            cc_out.append(cout_t)

        with tc.tile_pool(name="s_psum", bufs=2, space="PSUM") as spool, \
             tc.tile_pool(name="o_psum", bufs=4, space="PSUM") as opool, \
             tc.tile_pool(name="epool", bufs=4) as epool, \
             tc.tile_pool(name="rpool", bufs=2) as rpool, \
             tc.tile_pool(name="agp", bufs=2) as agp, \
             tc.tile_pool(name="ypool", bufs=3) as ypool:

            def emit_gather(chunk):
                nc.sync.dma_start(out=cc_in[chunk][:, :],
                                  in_=aT[:, ds(chunk * hseq, hseq)])
                nc.gpsimd.collective_compute(
                    "AllGather", mybir.AluOpType.bypass,
                    replica_groups=[list(range(n_cores))],
                    ins=[cc_in[chunk].opt()], outs=[cc_out[chunk].opt()])

            def emit_proj(chunk):
                ag = []
                for kb in range(KB):
                    t_ = agp.tile([P, hseq], BF16, tag=f"ag{kb}",
                                  name=f"ag{chunk}_{kb}")
                    nc.sync.dma_start(out=t_,
                                      in_=cc_out[chunk][ds(kb * P, P), :])
                    ag.append(t_)
                for nb in range(hseq // SUBW):
                    # projection PSUM borrows a score-pool slot
                    py = spool.tile([OC, SUBW], F32, tag="sAB",
                                    name=f"py{chunk}_{nb}")
                    for kb in range(KB):
                        nc.tensor.matmul(
                            py, lhsT=wt_sb[:, kb, :],
                            rhs=ag[kb][:, ts(nb, SUBW)],
                            start=(kb == 0), stop=(kb == KB - 1))
                    ysb = ypool.tile([OC, SUBW], F32, tag="ysb",
                                     name=f"ysb{chunk}_{nb}")
                    nc.vector.tensor_scalar_add(ysb, py, bias_sb)
                    nc.sync.dma_start(
                        out=out_d[:, ds(chunk * hseq + nb * SUBW, SUBW)],
                        in_=ysb)

            def emit_qk(ic, jb):
                sAB = spool.tile([P, 2 * IC], F32, tag="sAB",
                                 name=f"sAB_{ic}_{jb}")
                for h, cofs in ((0, 0), (1, IC)):
                    hd = ds(h * D, D)
                    for sub in range(NSUB):
                        nc.tensor.matmul(
                            sAB[:, ds(cofs + sub * SUBW, SUBW)],
                            lhsT=kT[hd, ts(jb, P)],
                            rhs=qT[hd, ds(ic * IC + sub * SUBW, SUBW)],
                            start=True, stop=True)
                return sAB

            step = 2 if PAIRED else 1
            pending_proj = []
            for icp in range(0, NIC, step):
                group = [icp + i for i in range(step)]
                otiles = {}
                for ic in group:
                    otiles[ic] = (
                        opool.tile([P, IC], F32, tag="o", name=f"oA_{ic}"),
                        opool.tile([P, IC], F32, tag="o", name=f"oB_{ic}"))
                s_cur = {ic: emit_qk(ic, 0) for ic in group}
                for jb in range(NT):
                    for ic in group:
                        eAB = epool.tile([P, 2 * IC], BF16, tag="e",
                                         name=f"e_{ic}_{jb}")
                        nc.scalar.activation(
                            eAB, s_cur[ic], mybir.ActivationFunctionType.Exp,
                            scale=1.0 / math.sqrt(D))
                        if jb + 1 < NT:
                            s_cur[ic] = emit_qk(ic, jb + 1)
                        oA, oB = otiles[ic]
                        for cofs, op_, v1 in ((0, oA, v1A), (IC, oB, v1B)):
                            for sub in range(NSUB):
                                nc.tensor.matmul(
                                    op_[:, ds(sub * SUBW, SUBW)],
                                    lhsT=v1[:, jb, :],
                                    rhs=eAB[:, ds(cofs + sub * SUBW, SUBW)],
                                    start=(jb == 0), stop=(jb == NT - 1))
                for ic in group:
                    oA, oB = otiles[ic]
                    for h, op_ in ((0, oA), (1, oB)):
                        rec_b = rpool.tile([D, IC], F32, tag="rec",
                                           name=f"rec_{ic}_{h}")
                        nc.vector.reciprocal(rec_b, op_[D:2 * D, :])
                        nc.vector.tensor_mul(
                            aT[ds(h * D, D), ds(ic * IC, IC)],
                            op_[0:D, :], rec_b)
                    if (ic + 1) % ICPC == 0:
                        emit_gather((ic + 1) // ICPC - 1)
                        pending_proj.append((ic + 1) // ICPC - 1)
                # run projections one group late: their matmuls only enter
                # the PE queue when the gathered data is already there, so
                # a slow AllGather never blocks attention matmuls.
                keep = max(1, step // max(ICPC, 1))
                while len(pending_proj) > keep:
                    emit_proj(pending_proj.pop(0))
            for chunk in pending_proj:
                emit_proj(chunk)

    nc.compile()
    return nc


def host_inputs(q, k, v, qk_scale, w_out, b_out, n_cores=N_CORES):
    """Shard + derive per-core input maps from the full problem inputs."""
    B, N, dim = q.shape
    assert B == 1 and dim == DIMF
    bf16 = ml_dtypes.bfloat16

    inv_freq = 1.0 / (10000.0 ** (np.arange(0, D, 2, dtype=np.float64) / D))
    t = np.arange(N, dtype=np.float64)
    freqs = np.outer(t, inv_freq)                       # [N, D/2]
    cos_e = np.repeat(np.cos(freqs), 2, axis=1)         # [N, D]
    ss_e = np.empty((N, D), dtype=np.float64)
    ss_e[:, 0::2] = -np.sin(freqs)
    ss_e[:, 1::2] = np.sin(freqs)
    sv = qk_scale.reshape(-1).astype(np.float64) ** 2   # [D]
    cosq = np.tile(cos_e, (1, HL)).astype(bf16)         # [N, FL]
    ssq = np.tile(ss_e, (1, HL)).astype(bf16)
    cosk = np.tile(cos_e * sv, (1, HL)).astype(bf16)
    ssk = np.tile(ss_e * sv, (1, HL)).astype(bf16)

    in_maps = []
    for c in range(n_cores):
        sl = slice(FL * c, FL * (c + 1))
        in_maps.append({
            "q": np.ascontiguousarray(q[0, :, sl]).astype(bf16),
            "k": np.ascontiguousarray(k[0, :, sl]).astype(bf16),
            "v": np.ascontiguousarray(v[0, :, sl]).astype(bf16),
            "cosq": cosq,
            "ssq": ssq,
            "cosk": cosk,
            "ssk": ssk,
            "wt": np.ascontiguousarray(w_out[sl, :].T).astype(bf16),
            "bias": np.ascontiguousarray(
                b_out[sl].reshape(OC, 1), dtype=np.float32),
        })
    return in_maps


def assemble_output(results, N=SEQ, n_cores=N_CORES):
    out = np.empty((1, N, DIMF), dtype=np.float32)
    for c in range(n_cores):
        out[0, :, FL * c:FL * (c + 1)] = results[c]["out"].T
    return out


_CACHE = {}


def kernel(q, k, v, qk_scale, w_out, b_out):
    from concourse.bass_utils import run_bass_kernel_spmd

    if "nc" not in _CACHE:
        _CACHE["nc"] = build_graph()
    nc = _CACHE["nc"]
    in_maps = host_inputs(q, k, v, qk_scale, w_out, b_out)
    res = run_bass_kernel_spmd(nc, in_maps, core_ids=list(range(N_CORES)))
    return assemble_output(res.results)
